# revision 27
# baseline (speedup 1.0000x reference)
"""Multi-head self-attention (B=2, S=2048, D=1024, H=16) on 8 Trainium2 cores.

Sharding: Megatron-style tensor parallelism on the head dimension.
Each core owns 2 heads (128 of the 1024 model dims):
  - Wq/Wk/Wv column-sharded: core c computes Q/K/V for dims [c*128,(c+1)*128)
  - attention for its 2 heads over both batches
  - Wo row-sharded: core c produces a partial output [4096, 1024]
  - host sums the 8 partials and adds bo.

All matmuls in bf16 (fp32 accumulate). Per-core device schedule is a
software pipeline keyed on the ScalarE exp throughput (the softmax exp of
S*S*2-heads elements is the per-core bottleneck at bf16 matmul rates):

  - x is loaded once into SBUF resident; Q/K/V projections are emitted as
    per-(projection, 512-token-chunk) "filler units" interleaved into the
    attention stream so the PE keeps ACT fed from ~t=4us onward.
  - scores are computed transposed, sT[k,q] = kT_tile.T @ qT_chunk, two
    heads row-tiled concurrently on the PE (K=64 each); exp on ScalarE
    reads the 2-bank fp32 PSUM tile [128,2,512] and writes bf16 at-tiles.
  - PV uses token-major V tiles [128, 64+ones+pad] (the ones column makes
    the PV matmul also produce the softmax normalizer as output row 64).
    V is projected o-major, then moved token-major by DMA xbar transposes
    (no PE transposes).
  - The kt loop of slot s also carries: PV second-half of slot s-1, PV
    first-half of slot s, the normalizer/ctx for s-1 (DVE+GpSimd), and the
    output projection of s-1 spread one matmul per kt (PSUM-copy paced).
"""

import os
import numpy as np
import ml_dtypes

import concourse.bass as bass
import concourse.tile as tile
from concourse import bacc, mybir
from concourse.bass_utils import run_bass_kernel_spmd

B, S, D = 2, 2048, 1024
H, DH = 16, 64
T = B * S                  # 4096 tokens total
N_CORES = 8
OPC = D // N_CORES         # 128 out dims per core
HPC = H // N_CORES         # 2 heads per core
NI = D // 128              # 8 contraction chunks of 128
TCH = 512                  # projection token chunk
NTCH = T // TCH            # 8
QCH = 512                  # attention q chunk
NQCH = S // QCH            # 4 per batch
NKT = S // 128             # 16 key tiles per batch
HW = DH + 2                # 66 cols per head in the v tile (data|ones|pad)
VW = HPC * HW              # 132

F32 = mybir.dt.float32
BF16 = mybir.dt.bfloat16
EXP = mybir.ActivationFunctionType.Exp

MM_NP = ml_dtypes.bfloat16

# PV lag: PV(s, kt) is emitted at slot-s iteration kt+LAG; the last LAG kts of
# slot s run during slot s+1 (and the pipeline tail shrinks with LAG).
LAG = 8

# filler schedule: slot -> {kt: (kind, chunk)}  kind in {"k","q","v","x"}
# chunk t covers tokens [t*512,(t+1)*512); b1 chunks are t=4..7.
# "k0b" is the second half of the K chunk 0 (first half runs in the prologue
# so the first scores fire early). "x" entries prefetch an x chunk on the
# gpsimd DMA queue.
FILLERS = {
    0: {0: ("k0b", 0), 1: [("x", 3), ("k", 1)], 3: [("x", 4), ("v", 0)],
        5: [("x", 5), ("k", 2)], 7: [("x", 6), ("v", 1)],
        9: [("x", 7), ("k", 3)], 11: ("v", 2), 13: ("q", 1), 15: ("v", 3)},
    1: {0: ("q", 2), 2: ("k", 4)},
    2: {0: ("q", 3), 2: ("k", 5)},
    3: {0: ("k", 6), 2: ("k", 7), 4: ("q", 4), 6: ("v", 4)},
    4: {0: ("v", 5), 2: ("v", 6), 4: ("q", 5), 6: ("v", 7)},
    5: {4: ("q", 6)},
    6: {4: ("q", 7)},
    7: {},
}


def _mha_kernel(tc, y, xT, wq, wk, wv, woT, bq, bk, bv):
    nc = tc.nc
    from contextlib import ExitStack

    with ExitStack() as ctx:
        _mha_kernel_inner(ctx, tc, y, xT, wq, wk, wv, woT, bq, bk, bv)


def _mha_kernel_inner(ctx, tc, y, xT, wq, wk, wv, woT, bq, bk, bv):
    nc = tc.nc
    pers = ctx.enter_context(tc.tile_pool(name="pers", bufs=1))

    x_sb = pers.tile([128, NI, T], BF16, tag="x")
    qT = pers.tile([128, T], BF16, tag="qT")
    kT = pers.tile([128, T], BF16, tag="kT")
    vT = pers.tile([128, T], BF16, tag="vT")
    # token-major V: [128 tok-in-tile, key-tile g, head-interleaved cols]
    vtk = pers.tile([128, B * NKT, VW], BF16, tag="vtk")
    wq_sb = pers.tile([128, NI, OPC], BF16, tag="wq")
    wk_sb = pers.tile([128, NI, OPC], BF16, tag="wk")
    wv_sb = pers.tile([128, NI, OPC], BF16, tag="wv")
    woT_sb = pers.tile([128, D], BF16, tag="wo")
    bq_sb = pers.tile([128, 1], F32, tag="bq")
    bk_sb = pers.tile([128, 1], F32, tag="bk")
    bv_sb = pers.tile([128, 1], F32, tag="bv")
    onepad = pers.tile([128, 2], BF16, tag="onepad")

    # weights/biases on the gpsimd DMA queue (sync queue carries x, v-transposes
    # and y); biases first since the first evacuations depend on them, then one
    # DMA per weight tensor (descriptor-issue time dominates small DMAs)
    nc.gpsimd.dma_start(bq_sb, bq)
    nc.gpsimd.dma_start(bk_sb, bk)
    nc.gpsimd.dma_start(bv_sb, bv)
    nc.gpsimd.dma_start(wk_sb, wk)
    nc.gpsimd.dma_start(wq_sb, wq)
    nc.gpsimd.dma_start(wv_sb, wv)
    nc.gpsimd.dma_start(woT_sb, woT)

    # constant ones/pad columns of vtk
    nc.vector.memset(onepad[:, 0:1], 1.0)
    nc.vector.memset(onepad[:, 1:2], 0.0)
    # pre-trigger the exp ACT table load so it doesn't stall the first real exp
    dummy = pers.tile([128, 2], BF16, tag="dummy")
    nc.scalar.activation(dummy, onepad, EXP, scale=1.0)
    onepad_b = bass.AP(
        tensor=onepad.tensor,
        offset=onepad.offset,
        ap=[onepad.ap[0], [0, B * NKT], onepad.ap[1]],
    )
    for h in range(HPC):
        nc.vector.tensor_copy(vtk[:, :, h * HW + DH : h * HW + DH + 2], onepad_b)

    psA = ctx.enter_context(tc.tile_pool(name="psA", bufs=2, space="PSUM"))
    psS = ctx.enter_context(tc.tile_pool(name="psS", bufs=2, space="PSUM"))
    psPV = ctx.enter_context(tc.tile_pool(name="psPV", bufs=1, space="PSUM"))
    atp = ctx.enter_context(tc.tile_pool(name="atp", bufs=10))
    smp = ctx.enter_context(tc.tile_pool(name="smp", bufs=2))
    yop = ctx.enter_context(tc.tile_pool(name="yop", bufs=3))
    vstg = ctx.enter_context(tc.tile_pool(name="vstg", bufs=2))

    x_loaded = set()
    pending_tr = []  # [t, age]: V chunks awaiting their DMA xbar transposes

    def emit_transposes(min_age=1):
        for ent in list(pending_tr):
            if ent[1] >= min_age:
                t = ent[0]
                sl = slice(t * TCH, (t + 1) * TCH)
                for h in range(HPC):
                    stg = vstg.tile(
                        [128, 4, DH], BF16, tag="vstg", name=f"vs{t}_{h}"
                    )
                    nc.sync.dma_start_transpose(stg, vT[h * DH : (h + 1) * DH, sl])
                    nc.vector.tensor_copy(
                        vtk[:, 4 * t : 4 * t + 4, h * HW : h * HW + DH], stg
                    )
                pending_tr.remove(ent)
            else:
                ent[1] += 1

    def load_x(t, eng=None):
        # per-(t,i) tiles so consumers wait per 128-d chunk, not the whole MB
        if t in x_loaded:
            return
        x_loaded.add(t)
        if eng is None:
            eng = nc.gpsimd
        sl = slice(t * TCH, (t + 1) * TCH)
        for i in range(NI):
            eng.dma_start(x_sb[:, i, sl], xT[i, :, sl])

    def proj_unit(kind, t, half=None):
        """One projection (q|k|v) for one 512-token chunk, + V transposes.
        half=0/1 restricts to a 256-token half chunk (early-pipeline split)."""
        if kind == "x":
            load_x(t)
            return
        assert t in x_loaded, (kind, t)
        if kind == "k0b":
            kind, half = "k", 1
        w_sb, b_sb, dst = {
            "q": (wq_sb, bq_sb, qT),
            "k": (wk_sb, bk_sb, kT),
            "v": (wv_sb, bv_sb, vT),
        }[kind]
        lo = t * TCH if half != 1 else t * TCH + TCH // 2
        hi = (t + 1) * TCH if half != 0 else t * TCH + TCH // 2
        ps = psA.tile([128, TCH], F32, tag="ps_x", name=f"ps_{kind}{t}")
        psw = ps[:, 0 : hi - lo]
        for i in range(NI):
            nc.tensor.matmul(
                psw,
                w_sb[:, i, :],
                x_sb[:, i, lo:hi],
                start=(i == 0),
                stop=(i == NI - 1),
            )
        sl = slice(lo, hi)
        nc.vector.tensor_scalar_add(dst[:, sl], psw, b_sb)
        if kind == "v":
            # defer the token-major transposes (sync-queue DMAs) so their
            # vT-write wait never blocks x loads queued behind them
            pending_tr.append([t, 0])

    def scores_exp(b, qc, kt):
        q0 = b * S + qc * QCH
        g = b * NKT + kt
        ps_s = psS.tile([128, HPC, QCH], F32, tag="ps_s")
        for h in range(HPC):
            hs = slice(h * DH, (h + 1) * DH)
            nc.tensor.matmul(
                ps_s[:, h, :],
                kT[hs, g * 128 : (g + 1) * 128],
                qT[hs, q0 : q0 + QCH],
                start=True,
                stop=True,
            )
        at = atp.tile([128, HPC, QCH], BF16, tag="at")
        nc.scalar.activation(at, ps_s, EXP, scale=0.125)
        return at

    def pv_mm(b, pvs, at_tiles, kt):
        g = b * NKT + kt
        for h in range(HPC):
            nc.tensor.matmul(
                pvs[h],
                vtk[:, g, h * HW : (h + 1) * HW],
                at_tiles[kt][:, h, :],
                start=(kt == 0),
                stop=(kt == NKT - 1),
            )

    def nrm_unit(pvs):
        """softmax normalize: ctx rows for head h = pv[0:64] * recip(pv[64])"""
        ctx_sb = smp.tile([128, QCH], BF16, tag="ctx")
        for h in range(HPC):
            rraw = smp.tile([1, QCH], F32, tag="rraw", name=f"rraw{h}")
            nc.vector.tensor_copy(rraw, pvs[h][DH : DH + 1, :])
            rrow = smp.tile([1, QCH], F32, tag="rrow", name=f"rrow{h}")
            nc.vector.reciprocal_approx_fast(rrow, rraw)
            nrm = smp.tile([DH, QCH], F32, tag="nrm", name=f"nrm{h}")
            nc.gpsimd.partition_broadcast(nrm, rrow)
            nc.vector.tensor_mul(
                ctx_sb[h * DH : (h + 1) * DH, :], pvs[h][0:DH, :], nrm
            )
        return ctx_sb

    def outproj_mm(state, j):
        """One output-projection matmul (of 8) for the given slot state."""
        ctx_sb, q0 = state["ctx"], state["q0"]
        t4, nch = j // 2, j % 2
        if nch == 0:
            state["yo"] = yop.tile([128, D], F32, tag="yo", name=f"yo{t4}")
        ps_o = psA.tile([128, 512], F32, tag="ps_x", name=f"ps_o{j}")
        nc.tensor.matmul(
            ps_o,
            ctx_sb[:, t4 * 128 : (t4 + 1) * 128],
            woT_sb[:, nch * 512 : (nch + 1) * 512],
            start=True,
            stop=True,
        )
        nc.vector.tensor_copy(state["yo"][:, nch * 512 : (nch + 1) * 512], ps_o)
        if nch == 1:
            r0 = q0 + t4 * 128
            nc.sync.dma_start(y[r0 : r0 + 128, :], state["yo"])

    # ---- software pipeline over 8 (b, qc) slots ----
    slots = [(b, qc) for b in range(B) for qc in range(NQCH)]

    # HAM warmup: a burst of tiny matmuls while the first x/weight DMAs are in
    # flight, so the PE clock is at 8/8 when real work starts.
    wrm = pers.tile([128, 64], BF16, tag="wrm")
    nc.vector.memset(wrm, 0.25)
    wps = psA.tile([128, TCH], F32, tag="ps_x", name="wps")
    for w in range(60):
        nc.tensor.matmul(
            wps[0:2, 0:64], onepad, wrm, start=True, stop=True,
            skip_group_check=True,
        )

    # prologue: x(t0)/x(t2) on the sync queue in parallel with weights on
    # gpsimd; then K (first half-chunk) and Q projections
    load_x(0, nc.sync)
    load_x(2, nc.sync)
    load_x(1, nc.gpsimd)
    proj_unit("k", 0, half=0)
    proj_unit("q", 0)

    at_live = {}      # slot -> list of at tiles
    pv_live = {}      # slot -> [pv0, pv1]
    op_state = {}     # slot -> {"ctx":, "q0":, "yo":}

    def emit_pv(s, kt_idx):
        if kt_idx == 0:
            pv_live[s] = [
                psPV.tile([HW, QCH], F32, tag=f"pv{h}", name=f"pv{h}")
                for h in range(HPC)
            ]
        pv_mm(slots[s][0], pv_live[s], at_live[s], kt_idx)

    def emit_nrm(s):
        bp, qp = slots[s]
        op_state[s] = {"ctx": nrm_unit(pv_live[s]), "q0": bp * S + qp * QCH}
        del at_live[s]

    for s, (b, qc) in enumerate(slots):
        at_live[s] = []
        for kt in range(NKT):
            at_live[s].append(scores_exp(b, qc, kt))
            if kt < LAG:
                if s >= 1:
                    emit_pv(s - 1, kt + NKT - LAG)
            else:
                emit_pv(s, kt - LAG)
            if kt == LAG - 1 and s >= 1:
                emit_nrm(s - 1)
            if s >= 1 and LAG + 1 <= kt <= 15:
                outproj_mm(op_state[s - 1], kt - LAG - 1)
                if kt == 15:
                    for j in range(kt - LAG, 8):
                        outproj_mm(op_state[s - 1], j)
                    del op_state[s - 1]
            f = FILLERS[s].get(kt)
            if f is not None:
                for unit in f if isinstance(f, list) else [f]:
                    proj_unit(*unit)
            emit_transposes()

    # tail: last LAG kts of the last slot's PV, its ctx and output projection
    emit_transposes(min_age=0)
    s = len(slots) - 1
    for kt_idx in range(NKT - LAG, NKT):
        emit_pv(s, kt_idx)
    emit_nrm(s)
    for j in range(8):
        outproj_mm(op_state[s], j)


_NC_CACHE = {}


def _build_nc(repeats=1):
    if repeats in _NC_CACHE:
        return _NC_CACHE[repeats]
    nc = bacc.Bacc("TRN2", target_bir_lowering=False, debug=False, num_devices=N_CORES)
    xT = nc.dram_tensor("xT", [NI, 128, T], BF16, kind="ExternalInput").ap()
    wq = nc.dram_tensor("wq", [128, NI, OPC], BF16, kind="ExternalInput").ap()
    wk = nc.dram_tensor("wk", [128, NI, OPC], BF16, kind="ExternalInput").ap()
    wv = nc.dram_tensor("wv", [128, NI, OPC], BF16, kind="ExternalInput").ap()
    woT = nc.dram_tensor("woT", [128, D], BF16, kind="ExternalInput").ap()
    bq = nc.dram_tensor("bq", [128, 1], F32, kind="ExternalInput").ap()
    bk = nc.dram_tensor("bk", [128, 1], F32, kind="ExternalInput").ap()
    bv = nc.dram_tensor("bv", [128, 1], F32, kind="ExternalInput").ap()
    y = nc.dram_tensor("y", [T, D], F32, kind="ExternalOutput").ap()
    with tile.TileContext(nc) as tc:
        for _ in range(repeats):
            _mha_kernel(tc, y, xT, wq, wk, wv, woT, bq, bk, bv)
    nc.compile()
    _NC_CACHE[repeats] = nc
    return nc


def _prep_in_maps(inputs):
    x = np.asarray(inputs["x"], np.float32)
    Wq = np.asarray(inputs["Wq"], np.float32)
    Wk = np.asarray(inputs["Wk"], np.float32)
    Wv = np.asarray(inputs["Wv"], np.float32)
    Wo = np.asarray(inputs["Wo"], np.float32)
    bq = np.asarray(inputs["bq"], np.float32)
    bk = np.asarray(inputs["bk"], np.float32)
    bv = np.asarray(inputs["bv"], np.float32)

    xT_np = np.ascontiguousarray(x.reshape(T, D).T).reshape(NI, 128, T).astype(MM_NP)

    def _w_slice(W, c):
        # [128(p), NI, OPC]: [p, i, o] = W[c*OPC+o, i*128+p]
        A = np.ascontiguousarray(W[c * OPC : (c + 1) * OPC, :].T)  # [D, OPC]
        return np.ascontiguousarray(A.reshape(NI, 128, OPC).transpose(1, 0, 2)).astype(
            MM_NP
        )

    in_maps = []
    for c in range(N_CORES):
        sl = slice(c * OPC, (c + 1) * OPC)
        in_maps.append(
            {
                "xT": xT_np,
                "wq": _w_slice(Wq, c),
                "wk": _w_slice(Wk, c),
                "wv": _w_slice(Wv, c),
                "woT": np.ascontiguousarray(Wo[:, sl].T).astype(MM_NP),
                "bq": bq[sl].reshape(OPC, 1).copy(),
                "bk": bk[sl].reshape(OPC, 1).copy(),
                "bv": bv[sl].reshape(OPC, 1).copy(),
            }
        )
    return in_maps


def kernel(**inputs) -> np.ndarray:
    nc = _build_nc()
    in_maps = _prep_in_maps(inputs)
    res = run_bass_kernel_spmd(nc, in_maps, core_ids=list(range(N_CORES)))
    bo = np.asarray(inputs["bo"], np.float32)
    y = np.zeros((T, D), np.float64)
    for c in range(N_CORES):
        y += res.results[c]["y"].astype(np.float64)
    y = (y + bo).astype(np.float32)
    return y.reshape(B, S, D)
